# revision 18
# baseline (speedup 1.0000x reference)
"""Multi-head attention (B=4, S=2048, E=1024, H=16, hd=64) on 8 TRN2 cores.

Sharding: core c -> batch b = c//2, head-half hh = c%2 (8 heads = 512 internal
dims).  Data parallel on B, tensor parallel on heads.  Each core computes a
partial out-projection for its batch; the host sums the two head-half partials
per batch and adds the (folded) output bias.

Device dataflow (bf16 matmuls, fp32 PSUM accumulation):
  - host pre-transposes q/k/v to (E, S) and casts to bf16 so the projection
    matmuls need no on-chip transpose:
      qhT (512 x 2048) = Wq_loc^T @ qT   [internal dims on partitions]
      khT likewise; vh (2048 x 512) via lhsT = vT slices, rhs = Wv_loc.
  - attention runs per head-PAIR (2g, 2g+1), per 512-query chunk:
      scoresT (keys x q) for A and B in one PSUM tile (row-group concurrent
      K=64 matmuls), one Exp over both (scale 1/8 pre-folded into qhT),
      then col-tiled concurrent AV matmuls (A -> partitions 0:64 via
      tile_position (0,0), B -> 64:128 via (0,64)) and concurrent ones-matmul
      denominators (rows 0 / 32 of a small PSUM tile).
  - softmax division: DVE fast-reciprocal of the denom rows, DMA round-trip
    through DRAM to broadcast across partitions, DVE multiply straight into
    attn_outT -- which is exactly the lhsT needed for the out-projection:
    po (q x E) = attn_outT^T @ Wo_loc.
"""

import math
import os
import sys
from contextlib import ExitStack

sys.path.insert(0, "/opt/trn_rl_repo")

import numpy as np
import ml_dtypes

import concourse.bass as bass
from concourse import bacc
import concourse.mybir as mybir
import concourse.tile as tile

F32 = mybir.dt.float32
BF16 = mybir.dt.bfloat16
AF = mybir.ActivationFunctionType

B, S, E = 4, 2048, 1024
H, HD = 16, 64
HLOC = 8          # heads per core
ILOC = HLOC * HD  # 512 internal dims per core
KT = E // 128     # 8 embed k-tiles
ST = S // 128     # 16 seq tiles
NCORES = 8
SCALE = 1.0 / math.sqrt(HD)  # 1/8


def build_nc():
    nc = bacc.Bacc()

    qT_d = nc.declare_dram_parameter("qT", [E, S], BF16, isOutput=False).ap()
    kT_d = nc.declare_dram_parameter("kT", [E, S], BF16, isOutput=False).ap()
    vT_d = nc.declare_dram_parameter("vT", [E, S], BF16, isOutput=False).ap()
    wq_d = nc.declare_dram_parameter("wq", [E, ILOC], BF16, isOutput=False).ap()
    wk_d = nc.declare_dram_parameter("wk", [E, ILOC], BF16, isOutput=False).ap()
    wv_d = nc.declare_dram_parameter("wv", [E, ILOC], BF16, isOutput=False).ap()
    wo_d = nc.declare_dram_parameter("wo", [ILOC, E], BF16, isOutput=False).ap()
    bq_d = nc.declare_dram_parameter("bq", [128, 4], F32, isOutput=False).ap()
    bk_d = nc.declare_dram_parameter("bk", [128, 4], F32, isOutput=False).ap()
    out_d = nc.declare_dram_parameter("out", [S, E], F32, isOutput=True).ap()
    dscr = nc.dram_tensor("dscratch", [32, 512], F32).ap()

    with tile.TileContext(nc) as tc, ExitStack() as ctx:
        # ---- persistent pools ----
        psum = ctx.enter_context(tc.tile_pool(name="psum", bufs=2, space="PSUM"))
        qhT_pool = ctx.enter_context(tc.tile_pool(name="qhT", bufs=4))
        khT_pool = ctx.enter_context(tc.tile_pool(name="khT", bufs=4))
        vh_pool = ctx.enter_context(tc.tile_pool(name="vh", bufs=ST))
        bias_pool = ctx.enter_context(tc.tile_pool(name="bias", bufs=1))

        qhT = [qhT_pool.tile([128, S], BF16, tag="qhT", name=f"qhT{i}")
               for i in range(4)]
        khT = [khT_pool.tile([128, S], BF16, tag="khT", name=f"khT{i}")
               for i in range(4)]
        vh = [vh_pool.tile([128, HLOC * 65], BF16, tag="vh", name=f"vh{i}")
              for i in range(ST)]

        bq_t = bias_pool.tile([128, 4], F32, tag="bq")
        bk_t = bias_pool.tile([128, 4], F32, tag="bk")
        nc.sync.dma_start(bq_t[:], bq_d[:])
        nc.sync.dma_start(bk_t[:], bk_d[:])

        # ================= Phase A: projections =================
        with tc.tile_pool(name="w_in", bufs=3) as wpool, \
             tc.tile_pool(name="stage", bufs=9) as stage_pool:
            wq_t = wpool.tile([128, KT, ILOC], BF16, tag="w")
            wk_t = wpool.tile([128, KT, ILOC], BF16, tag="w")
            wv_t = wpool.tile([128, KT, ILOC], BF16, tag="w")
            nc.sync.dma_start(wq_t[:], wq_d.rearrange("(k p) n -> p k n", p=128))
            nc.sync.dma_start(wk_t[:], wk_d.rearrange("(k p) n -> p k n", p=128))
            nc.sync.dma_start(wv_t[:], wv_d.rearrange("(k p) n -> p k n", p=128))

            # vh first (attention needs all of it), seq on partitions
            stg = []
            for kk in range(KT):
                t = stage_pool.tile([128, S], BF16, tag="stage")
                nc.sync.dma_start(t[:], vT_d[kk * 128:(kk + 1) * 128, :])
                stg.append(t)
            for st in range(ST):
                ps = psum.tile([128, 1024], F32, tag="ps")
                for kk in range(KT):
                    nc.tensor.matmul(
                        ps[:, 0:512],
                        lhsT=stg[kk][:, st * 128:(st + 1) * 128],
                        rhs=wv_t[:, kk, :],
                        start=(kk == 0), stop=(kk == KT - 1),
                    )
                pin = ps[:, 0:512].rearrange("p (a b x) -> p a b x", b=2, x=64)
                pout = vh[st].rearrange("p (a c) -> p a c", c=130)
                nc.vector.tensor_copy(pout[:, :, 0:64], pin[:, :, 0, :])
                nc.vector.tensor_copy(pout[:, :, 65:129], pin[:, :, 1, :])
                ones = vh[st].rearrange("p (h x) -> p h x", x=65)[:, :, 64:65]
                nc.vector.memset(ones, 1.0)

            # khT then qhT: internal dims on partitions, seq on free
            for src_d, w_t, dst, b_t, scale in (
                (kT_d, wk_t, khT, bk_t, 1.0),
                (qT_d, wq_t, qhT, bq_t, SCALE),
            ):
                stg = []
                for kk in range(KT):
                    t = stage_pool.tile([128, S], BF16, tag="stage")
                    nc.sync.dma_start(t[:], src_d[kk * 128:(kk + 1) * 128, :])
                    stg.append(t)
                for m in range(4):
                    for half in range(2):
                        cols = slice(half * 1024, (half + 1) * 1024)
                        ps = psum.tile([128, 1024], F32, tag="ps")
                        for c in range(2):
                            lo = half * 1024 + c * 512
                            for kk in range(KT):
                                nc.tensor.matmul(
                                    ps[:, c * 512:(c + 1) * 512],
                                    lhsT=w_t[:, kk, m * 128:(m + 1) * 128],
                                    rhs=stg[kk][:, lo:lo + 512],
                                    start=(kk == 0), stop=(kk == KT - 1),
                                )
                        nc.vector.tensor_scalar(
                            out=dst[m][:, cols], in0=ps[:],
                            scalar1=scale, scalar2=b_t[:, m:m + 1],
                            op0=mybir.AluOpType.mult, op1=mybir.AluOpType.add,
                        )

        # ================= Phase B: attention =================
        with tc.tile_pool(name="wo", bufs=1) as wo_pool, \
             tc.tile_pool(name="exp", bufs=3) as exp_pool, \
             tc.tile_pool(name="attnT", bufs=4) as attnT_pool, \
             tc.tile_pool(name="avp", bufs=4, space="PSUM") as av_pool, \
             tc.tile_pool(name="small", bufs=4) as small_pool, \
             tc.tile_pool(name="bcb", bufs=4) as bc_pool, \
             tc.tile_pool(name="tmpp", bufs=2) as tmp_pool, \
             tc.tile_pool(name="outbuf", bufs=2) as out_pool:
            wo_t = wo_pool.tile([128, 4, E], BF16, tag="wo")
            nc.sync.dma_start(wo_t[:], wo_d.rearrange("(i p) n -> p i n", p=128))

            attnT = [attnT_pool.tile([128, S], BF16, tag="attnT",
                                     name=f"attnT{i}") for i in range(4)]

            for qc in range(4):             # 512-query chunks
                for g in range(4):          # head pair (2g, 2g+1)
                    qlo = qc * 512
                    qcols = slice(qlo, qlo + 512)
                    avA = av_pool.tile([65, 512], F32, tag="av", name="avA")
                    avB = av_pool.tile([65, 512], F32, tag="av", name="avB")
                    for kt in range(ST):
                        sc = psum.tile([128, 1024], F32, tag="ps", name="sc")
                        nc.tensor.matmul(
                            sc[:, 0:512],
                            lhsT=khT[g][0:64, kt * 128:(kt + 1) * 128],
                            rhs=qhT[g][0:64, qcols],
                            start=True, stop=True,
                        )
                        nc.tensor.matmul(
                            sc[:, 512:1024],
                            lhsT=khT[g][64:128, kt * 128:(kt + 1) * 128],
                            rhs=qhT[g][64:128, qcols],
                            start=True, stop=True,
                        )
                        ex = exp_pool.tile([128, 1024], BF16, tag="exp", name="ex")
                        nc.scalar.activation(ex[:], sc[:], AF.Exp)
                        first, last = (kt == 0), (kt == ST - 1)
                        hA, hB = 2 * g, 2 * g + 1
                        nc.tensor.matmul(
                            avA[0:65, :],
                            lhsT=vh[kt][:, hA * 65:hA * 65 + 65],
                            rhs=ex[:, 0:512],
                            start=first, stop=last,
                        )
                        nc.tensor.matmul(
                            avB[0:65, :],
                            lhsT=vh[kt][:, hB * 65:hB * 65 + 65],
                            rhs=ex[:, 512:1024],
                            start=first, stop=last,
                        )
                    # softmax division for both heads of the pair
                    idx = (g * 4 + qc) * 2
                    dsbA = small_pool.tile([65, 512], F32, tag="dsb", name="dsbA")
                    dsbB = small_pool.tile([65, 512], F32, tag="dsb", name="dsbB")
                    nc.vector.tensor_copy(dsbA[64:65, :], avA[64:65, :])
                    nc.vector.tensor_copy(dsbB[64:65, :], avB[64:65, :])
                    nc.sync.dma_start(dscr[idx:idx + 1, :], dsbA[64:65, :])
                    nc.sync.dma_start(dscr[idx + 1:idx + 2, :], dsbB[64:65, :])
                    bcA = bc_pool.tile([64, 512], F32, tag="bc", name="bcA")
                    bcB = bc_pool.tile([64, 512], F32, tag="bc", name="bcB")
                    nc.sync.dma_start(
                        bcA[:].rearrange("p (o n) -> p o n", o=1),
                        dscr[idx, :].partition_broadcast(64))
                    nc.sync.dma_start(
                        bcB[:].rearrange("p (o n) -> p o n", o=1),
                        dscr[idx + 1, :].partition_broadcast(64))
                    nc.vector.reciprocal_approx_fast(bcA[:], bcA[:])
                    nc.vector.reciprocal_approx_fast(bcB[:], bcB[:])
                    nc.vector.tensor_mul(attnT[g][0:64, qcols],
                                         avA[0:64, :], bcA[:])
                    tmp = tmp_pool.tile([64, 512], BF16, tag="tmp", name="tmp")
                    nc.vector.tensor_mul(tmp[:], avB[0:64, :], bcB[:])
                    nc.sync.dma_start(attnT[g][64:128, qcols], tmp[:])

                # out-projection for this query chunk (4 q-tiles)
                for qt in range(qc * 4, qc * 4 + 4):
                    po = psum.tile([128, 1024], F32, tag="ps", name="po")
                    for c in range(2):
                        for it in range(4):
                            nc.tensor.matmul(
                                po[:, c * 512:(c + 1) * 512],
                                lhsT=attnT[it][:, qt * 128:(qt + 1) * 128],
                                rhs=wo_t[:, it, c * 512:(c + 1) * 512],
                                start=(it == 0), stop=(it == 3),
                            )
                    ot = out_pool.tile([128, 1024], F32, tag="ot", name="ot")
                    nc.vector.tensor_copy(ot[:], po[:])
                    nc.sync.dma_start(out_d[qt * 128:(qt + 1) * 128, :], ot[:])

    nc.finalize()
    return nc


def make_in_maps(q, k, v, Wq, bq, Wk, bk, Wv, bv, Wo, bo):
    """Per-core input dicts + the folded host-side bias."""
    bf = ml_dtypes.bfloat16
    qT = [np.ascontiguousarray(q[b].T).astype(bf) for b in range(B)]
    kT = [np.ascontiguousarray(k[b].T).astype(bf) for b in range(B)]
    vT = [np.ascontiguousarray(v[b].T).astype(bf) for b in range(B)]
    in_maps = []
    for c in range(NCORES):
        b, hh = divmod(c, 2)
        isl = slice(hh * ILOC, (hh + 1) * ILOC)
        bq_loc = np.ascontiguousarray(
            (bq[isl] * SCALE).reshape(4, 128).T)
        bk_loc = np.ascontiguousarray(bk[isl].reshape(4, 128).T)
        in_maps.append({
            "qT": qT[b], "kT": kT[b], "vT": vT[b],
            "wq": np.ascontiguousarray(Wq[:, isl]).astype(bf),
            "wk": np.ascontiguousarray(Wk[:, isl]).astype(bf),
            "wv": np.ascontiguousarray(Wv[:, isl]).astype(bf),
            "wo": np.ascontiguousarray(Wo[isl, :]).astype(bf),
            "bq": bq_loc, "bk": bk_loc,
        })
    bo_eff = (bo + bv @ Wo).astype(np.float32)
    return in_maps, bo_eff


_NC_CACHE = None


def kernel(q, k, v, Wq, bq, Wk, bk, Wv, bv, Wo, bo):
    global _NC_CACHE
    from concourse.bass_utils import run_bass_kernel_spmd

    if _NC_CACHE is None:
        _NC_CACHE = build_nc()
    nc = _NC_CACHE
    in_maps, bo_eff = make_in_maps(q, k, v, Wq, bq, Wk, bk, Wv, bv, Wo, bo)
    res = run_bass_kernel_spmd(nc, in_maps, list(range(NCORES)))
    out = np.empty((B, S, E), np.float32)
    for b in range(B):
        out[b] = res.results[2 * b]["out"] + res.results[2 * b + 1]["out"] + bo_eff
    return out


# revision 19
# speedup vs baseline: 1.1619x; 1.1619x over previous
"""Multi-head attention (B=4, S=2048, E=1024, H=16, hd=64) on 8 TRN2 cores.

Sharding: core c -> batch b = c//2, head-half hh = c%2 (8 heads = 512 internal
dims).  Data parallel on B, tensor parallel on heads.  Each core computes a
partial out-projection for its batch; the host sums the two head-half partials
per batch and adds the (folded) output bias.

Device dataflow (bf16 matmuls, fp32 PSUM accumulation):
  - host pre-transposes q/k/v to (E, S) and casts to bf16 so the projection
    matmuls need no on-chip transpose:
      qhT (512 x 2048) = Wq_loc^T @ qT   [internal dims on partitions]
      khT likewise; vh (2048 x 512) via lhsT = vT slices, rhs = Wv_loc.
  - attention runs per head-PAIR (2g, 2g+1), per 512-query chunk:
      scoresT (keys x q) for A and B in one PSUM tile (row-group concurrent
      K=64 matmuls), one Exp over both (scale 1/8 pre-folded into qhT),
      then col-tiled concurrent AV matmuls (A -> partitions 0:64 via
      tile_position (0,0), B -> 64:128 via (0,64)) and concurrent ones-matmul
      denominators (rows 0 / 32 of a small PSUM tile).
  - softmax division: DVE fast-reciprocal of the denom rows, DMA round-trip
    through DRAM to broadcast across partitions, DVE multiply straight into
    attn_outT -- which is exactly the lhsT needed for the out-projection:
    po (q x E) = attn_outT^T @ Wo_loc.
"""

import math
import os
import sys
from contextlib import ExitStack

sys.path.insert(0, "/opt/trn_rl_repo")

import numpy as np
import ml_dtypes

import concourse.bass as bass
from concourse import bacc
import concourse.mybir as mybir
import concourse.tile as tile

F32 = mybir.dt.float32
BF16 = mybir.dt.bfloat16
AF = mybir.ActivationFunctionType

B, S, E = 4, 2048, 1024
H, HD = 16, 64
HLOC = 8          # heads per core
ILOC = HLOC * HD  # 512 internal dims per core
KT = E // 128     # 8 embed k-tiles
ST = S // 128     # 16 seq tiles
NCORES = 8
SCALE = 1.0 / math.sqrt(HD)  # 1/8


def build_nc():
    nc = bacc.Bacc()

    qT_d = nc.declare_dram_parameter("qT", [E, S], BF16, isOutput=False).ap()
    kT_d = nc.declare_dram_parameter("kT", [E, S], BF16, isOutput=False).ap()
    vT_d = nc.declare_dram_parameter("vT", [E, S], BF16, isOutput=False).ap()
    wq_d = nc.declare_dram_parameter("wq", [E, ILOC], BF16, isOutput=False).ap()
    wk_d = nc.declare_dram_parameter("wk", [E, ILOC], BF16, isOutput=False).ap()
    wv_d = nc.declare_dram_parameter("wv", [E, ILOC], BF16, isOutput=False).ap()
    wo_d = nc.declare_dram_parameter("wo", [ILOC, E], BF16, isOutput=False).ap()
    bq_d = nc.declare_dram_parameter("bq", [128, 4], F32, isOutput=False).ap()
    bk_d = nc.declare_dram_parameter("bk", [128, 4], F32, isOutput=False).ap()
    out_d = nc.declare_dram_parameter("out", [S, E], F32, isOutput=True).ap()
    dscr = nc.dram_tensor("dscratch", [32, 512], F32).ap()

    with tile.TileContext(nc) as tc, ExitStack() as ctx:
        # ---- persistent pools ----
        psum = ctx.enter_context(tc.tile_pool(name="psum", bufs=2, space="PSUM"))
        qhT_pool = ctx.enter_context(tc.tile_pool(name="qhT", bufs=4))
        khT_pool = ctx.enter_context(tc.tile_pool(name="khT", bufs=4))
        vh_pool = ctx.enter_context(tc.tile_pool(name="vh", bufs=ST))
        bias_pool = ctx.enter_context(tc.tile_pool(name="bias", bufs=1))

        qhT = [qhT_pool.tile([128, S], BF16, tag="qhT", name=f"qhT{i}")
               for i in range(4)]
        khT = [khT_pool.tile([128, S], BF16, tag="khT", name=f"khT{i}")
               for i in range(4)]
        vh = [vh_pool.tile([128, HLOC * 65], BF16, tag="vh", name=f"vh{i}")
              for i in range(ST)]

        bq_t = bias_pool.tile([128, 4], F32, tag="bq")
        bk_t = bias_pool.tile([128, 4], F32, tag="bk")
        nc.sync.dma_start(bq_t[:], bq_d[:])
        nc.sync.dma_start(bk_t[:], bk_d[:])

        # ================= Phase A: projections =================
        with tc.tile_pool(name="w_in", bufs=3) as wpool, \
             tc.tile_pool(name="stage", bufs=17) as stage_pool:
            wq_t = wpool.tile([128, KT, ILOC], BF16, tag="w")
            wk_t = wpool.tile([128, KT, ILOC], BF16, tag="w")
            wv_t = wpool.tile([128, KT, ILOC], BF16, tag="w")
            nc.sync.dma_start(wq_t[:], wq_d.rearrange("(k p) n -> p k n", p=128))
            nc.sync.dma_start(wk_t[:], wk_d.rearrange("(k p) n -> p k n", p=128))
            nc.sync.dma_start(wv_t[:], wv_d.rearrange("(k p) n -> p k n", p=128))

            # vh first (attention needs all of it), seq on partitions
            stg = []
            for kk in range(KT):
                t = stage_pool.tile([128, S], BF16, tag="stage")
                nc.sync.dma_start(t[:], vT_d[kk * 128:(kk + 1) * 128, :])
                stg.append(t)
            for st in range(ST):
                ps = psum.tile([128, 1024], F32, tag="ps")
                for kk in range(KT):
                    nc.tensor.matmul(
                        ps[:, 0:512],
                        lhsT=stg[kk][:, st * 128:(st + 1) * 128],
                        rhs=wv_t[:, kk, :],
                        start=(kk == 0), stop=(kk == KT - 1),
                    )
                pin = ps[:, 0:512].rearrange("p (a b x) -> p a b x", b=2, x=64)
                pout = vh[st].rearrange("p (a c) -> p a c", c=130)
                nc.vector.tensor_copy(pout[:, :, 0:64], pin[:, :, 0, :])
                nc.vector.tensor_copy(pout[:, :, 65:129], pin[:, :, 1, :])
                ones = vh[st].rearrange("p (h x) -> p h x", x=65)[:, :, 64:65]
                nc.vector.memset(ones, 1.0)

            # khT / qhT interleaved by m-tile so attention pair g can start
            # as soon as m-tile g of both is projected
            stg_k, stg_q = [], []
            for stg, src_d in ((stg_k, kT_d), (stg_q, qT_d)):
                for kk in range(KT):
                    t = stage_pool.tile([128, S], BF16, tag="stage")
                    nc.sync.dma_start(t[:], src_d[kk * 128:(kk + 1) * 128, :])
                    stg.append(t)
            for m in range(4):
                for stg, w_t, dst, b_t, scale in (
                    (stg_k, wk_t, khT, bk_t, 1.0),
                    (stg_q, wq_t, qhT, bq_t, SCALE),
                ):
                    for half in range(2):
                        cols = slice(half * 1024, (half + 1) * 1024)
                        ps = psum.tile([128, 1024], F32, tag="ps")
                        for c in range(2):
                            lo = half * 1024 + c * 512
                            for kk in range(KT):
                                nc.tensor.matmul(
                                    ps[:, c * 512:(c + 1) * 512],
                                    lhsT=w_t[:, kk, m * 128:(m + 1) * 128],
                                    rhs=stg[kk][:, lo:lo + 512],
                                    start=(kk == 0), stop=(kk == KT - 1),
                                )
                        nc.vector.tensor_scalar(
                            out=dst[m][:, cols], in0=ps[:],
                            scalar1=scale, scalar2=b_t[:, m:m + 1],
                            op0=mybir.AluOpType.mult, op1=mybir.AluOpType.add,
                        )

        # ================= Phase B: attention =================
        with tc.tile_pool(name="wo", bufs=1) as wo_pool, \
             tc.tile_pool(name="exp", bufs=3) as exp_pool, \
             tc.tile_pool(name="attnT", bufs=4) as attnT_pool, \
             tc.tile_pool(name="avp", bufs=4, space="PSUM") as av_pool, \
             tc.tile_pool(name="small", bufs=4) as small_pool, \
             tc.tile_pool(name="bcb", bufs=4) as bc_pool, \
             tc.tile_pool(name="tmpp", bufs=2) as tmp_pool, \
             tc.tile_pool(name="outbuf", bufs=2) as out_pool:
            wo_t = wo_pool.tile([128, 4, E], BF16, tag="wo")
            nc.sync.dma_start(wo_t[:], wo_d.rearrange("(i p) n -> p i n", p=128))

            attnT = [attnT_pool.tile([128, S], BF16, tag="attnT",
                                     name=f"attnT{i}") for i in range(4)]

            for g in range(4):              # head pair (2g, 2g+1)
                for qc in range(4):         # 512-query chunks
                    qlo = qc * 512
                    qcols = slice(qlo, qlo + 512)
                    avA = av_pool.tile([65, 512], F32, tag="av", name="avA")
                    avB = av_pool.tile([65, 512], F32, tag="av", name="avB")
                    for kt in range(ST):
                        sc = psum.tile([128, 1024], F32, tag="ps", name="sc")
                        nc.tensor.matmul(
                            sc[:, 0:512],
                            lhsT=khT[g][0:64, kt * 128:(kt + 1) * 128],
                            rhs=qhT[g][0:64, qcols],
                            start=True, stop=True,
                        )
                        nc.tensor.matmul(
                            sc[:, 512:1024],
                            lhsT=khT[g][64:128, kt * 128:(kt + 1) * 128],
                            rhs=qhT[g][64:128, qcols],
                            start=True, stop=True,
                        )
                        ex = exp_pool.tile([128, 1024], BF16, tag="exp", name="ex")
                        nc.scalar.activation(ex[:], sc[:], AF.Exp)
                        first, last = (kt == 0), (kt == ST - 1)
                        hA, hB = 2 * g, 2 * g + 1
                        nc.tensor.matmul(
                            avA[0:65, :],
                            lhsT=vh[kt][:, hA * 65:hA * 65 + 65],
                            rhs=ex[:, 0:512],
                            start=first, stop=last,
                        )
                        nc.tensor.matmul(
                            avB[0:65, :],
                            lhsT=vh[kt][:, hB * 65:hB * 65 + 65],
                            rhs=ex[:, 512:1024],
                            start=first, stop=last,
                        )
                    # softmax division for both heads of the pair
                    idx = (g * 4 + qc) * 2
                    dsbA = small_pool.tile([65, 512], F32, tag="dsb", name="dsbA")
                    dsbB = small_pool.tile([65, 512], F32, tag="dsb", name="dsbB")
                    nc.vector.tensor_copy(dsbA[64:65, :], avA[64:65, :])
                    nc.vector.tensor_copy(dsbB[64:65, :], avB[64:65, :])
                    nc.sync.dma_start(dscr[idx:idx + 1, :], dsbA[64:65, :])
                    nc.sync.dma_start(dscr[idx + 1:idx + 2, :], dsbB[64:65, :])
                    bcA = bc_pool.tile([64, 512], F32, tag="bc", name="bcA")
                    bcB = bc_pool.tile([64, 512], F32, tag="bc", name="bcB")
                    nc.sync.dma_start(
                        bcA[:].rearrange("p (o n) -> p o n", o=1),
                        dscr[idx, :].partition_broadcast(64))
                    nc.sync.dma_start(
                        bcB[:].rearrange("p (o n) -> p o n", o=1),
                        dscr[idx + 1, :].partition_broadcast(64))
                    nc.vector.reciprocal_approx_fast(bcA[:], bcA[:])
                    nc.vector.reciprocal_approx_fast(bcB[:], bcB[:])
                    nc.vector.tensor_mul(attnT[g][0:64, qcols],
                                         avA[0:64, :], bcA[:])
                    tmp = tmp_pool.tile([64, 512], BF16, tag="tmp", name="tmp")
                    nc.vector.tensor_mul(tmp[:], avB[0:64, :], bcB[:])
                    nc.sync.dma_start(attnT[g][64:128, qcols], tmp[:])

            # ================= Phase C: out-projection =================
            for qt in range(ST):
                po = psum.tile([128, 1024], F32, tag="ps", name="po")
                for c in range(2):
                    for it in range(4):
                        nc.tensor.matmul(
                            po[:, c * 512:(c + 1) * 512],
                            lhsT=attnT[it][:, qt * 128:(qt + 1) * 128],
                            rhs=wo_t[:, it, c * 512:(c + 1) * 512],
                            start=(it == 0), stop=(it == 3),
                        )
                ot = out_pool.tile([128, 1024], F32, tag="ot", name="ot")
                nc.vector.tensor_copy(ot[:], po[:])
                nc.sync.dma_start(out_d[qt * 128:(qt + 1) * 128, :], ot[:])

    nc.finalize()
    return nc


def make_in_maps(q, k, v, Wq, bq, Wk, bk, Wv, bv, Wo, bo):
    """Per-core input dicts + the folded host-side bias."""
    bf = ml_dtypes.bfloat16
    qT = [np.ascontiguousarray(q[b].T).astype(bf) for b in range(B)]
    kT = [np.ascontiguousarray(k[b].T).astype(bf) for b in range(B)]
    vT = [np.ascontiguousarray(v[b].T).astype(bf) for b in range(B)]
    in_maps = []
    for c in range(NCORES):
        b, hh = divmod(c, 2)
        isl = slice(hh * ILOC, (hh + 1) * ILOC)
        bq_loc = np.ascontiguousarray(
            (bq[isl] * SCALE).reshape(4, 128).T)
        bk_loc = np.ascontiguousarray(bk[isl].reshape(4, 128).T)
        in_maps.append({
            "qT": qT[b], "kT": kT[b], "vT": vT[b],
            "wq": np.ascontiguousarray(Wq[:, isl]).astype(bf),
            "wk": np.ascontiguousarray(Wk[:, isl]).astype(bf),
            "wv": np.ascontiguousarray(Wv[:, isl]).astype(bf),
            "wo": np.ascontiguousarray(Wo[isl, :]).astype(bf),
            "bq": bq_loc, "bk": bk_loc,
        })
    bo_eff = (bo + bv @ Wo).astype(np.float32)
    return in_maps, bo_eff


_NC_CACHE = None


def kernel(q, k, v, Wq, bq, Wk, bk, Wv, bv, Wo, bo):
    global _NC_CACHE
    from concourse.bass_utils import run_bass_kernel_spmd

    if _NC_CACHE is None:
        _NC_CACHE = build_nc()
    nc = _NC_CACHE
    in_maps, bo_eff = make_in_maps(q, k, v, Wq, bq, Wk, bk, Wv, bv, Wo, bo)
    res = run_bass_kernel_spmd(nc, in_maps, list(range(NCORES)))
    out = np.empty((B, S, E), np.float32)
    for b in range(B):
        out[b] = res.results[2 * b]["out"] + res.results[2 * b + 1]["out"] + bo_eff
    return out
